# revision 23
# baseline (speedup 1.0000x reference)
"""AngularPenaltySMLoss (CosFace) on 8 TRN2 NeuronCores.

Strategy: data-parallel over the batch N=4096. Each core owns 512 samples
and streams the FULL weight matrix W [100000, 512] (fp8, 51.2 MB) from
HBM; 100000 = 195*512 + 160, so there is no class padding at all and no
per-sample denominator collective. Per core:
  - logits tile [128 n x 512 c] = fp8 DoubleRow matmul of xT (stationary)
    against W^T (moving), K=512 contracted in 2 accumulating PSUM steps;
    c-tiles processed in chunks of 4 (one 4-bank PSUM group per chunk)
  - one wide ScalarE Exp activation per PSUM group with per-partition
    scale a[n] = S/||x_n|| (folds the L2 normalization and the S factor
    into the activation) and the fused row-sum accumulator. Every Nth
    group goes to VectorE instead via the Schraudolph fast-exp bit trick
    so ScalarE stays below the PE roofline.
  - 1/||x|| computed as Exp(-0.5*Ln(ss)) so ScalarE only ever needs the
    natural_log_exp table set (no table thrashing with Sqrt).
  - target logits from host-gathered W[labels] rows (data movement only):
    DVE multiply + row-reduce, 4 tiles, done during the first chunks.
  - per-core partial sum of (log(den) - S*tgt) -> AllReduce [128,1]
    (512 B) -> partition-sum via ones-matmul -> mean + S*margin.

xT and W are pre-cast on the host (pure dtype/layout prep) to fp8 to
halve DMA volume; x/W[labels] stay bf16 for the norm/target paths.
"""

import ml_dtypes
import numpy as np

from concourse import bacc, mybir, tile
from concourse.bass_utils import run_bass_kernel_spmd

N, D, C = 4096, 512, 100000
N_CORES = 8
N_SHARD = N // N_CORES          # 512 samples per core
CT = 512                        # class-tile width (one PSUM bank of f32)
S = 30.0
SM = 10.5                       # S * margin(0.35)

# Schraudolph fast-exp constants (DVE offload): exp(x) ~= bitcast_f32(
# int32(x * 2^23/ln2 + (127*2^23 - C))), C=486411 zeroes the mean error
EXP_A = float(2 ** 23 / np.log(2))
EXP_B = float(1065353216 - 486411)

f32 = mybir.dt.float32
bf16 = mybir.dt.bfloat16
fp8 = mybir.dt.float8e4
i32 = mybir.dt.int32
np_bf16 = ml_dtypes.bfloat16
np_fp8 = mybir.dt.np(mybir.dt.float8e4)
AF = mybir.ActivationFunctionType
ALU = mybir.AluOpType
AX = mybir.AxisListType


def build(n=N_SHARD, d=D, c=C, ct=CT, n_cores=N_CORES,
          dve_every=5, prefetch_chunks=4):
    ni = n // 128                 # 4 row-tiles per core
    nk8 = d // 256                # 2 DoubleRow contraction steps
    # class tiles: full 512-wide plus one 160-wide tail (160 % 16 == 0)
    tile_widths = [ct] * (c // ct)
    if c % ct:
        assert (c % ct) % 16 == 0 and (c % ct) >= 128
        tile_widths.append(c % ct)
    # chunks of up to 4 tiles -> one PSUM group each
    chunks = []                   # list of (col_offset, [widths])
    off = 0
    while off < len(tile_widths):
        ws = tile_widths[off:off + 4]
        chunks.append((off, ws))
        off += len(ws)
    n_chunks = len(chunks)
    psg_w = 4 * ct

    nc = bacc.Bacc("TRN2", target_bir_lowering=False, debug=False,
                   num_devices=n_cores)
    x_nat = nc.dram_tensor("x_nat", [n, d], bf16, kind="ExternalInput").ap()
    xtb_d = nc.dram_tensor("xtb", [d, n], fp8, kind="ExternalInput").ap()
    wl = nc.dram_tensor("wl", [n, d], bf16, kind="ExternalInput").ap()
    wt = nc.dram_tensor("wt", [d, c], fp8, kind="ExternalInput").ap()
    out = nc.dram_tensor("out", [1, 1], f32, kind="ExternalOutput").ap()

    with tile.TileContext(nc) as tc:
        with (
            tc.tile_pool(name="persist", bufs=1) as pp,
            tc.tile_pool(name="stage", bufs=2) as sp,
            tc.tile_pool(name="wbuf", bufs=prefetch_chunks * nk8) as wbp,
            tc.tile_pool(name="scr", bufs=2) as scp,
            tc.tile_pool(name="dram", bufs=1, space="DRAM") as dp,
        ):
            xtb = [pp.tile([128, 2, n], fp8, tag=f"xtb{g}",
                           name=f"xtbs{g}") for g in range(nk8)]
            xa = [pp.tile([128, d], bf16, tag=f"xa{i}", name=f"xa{i}")
                  for i in range(ni)]
            parts = pp.tile([128, ni * n_chunks], f32, tag="parts",
                            name="parts")
            ss = pp.tile([128, ni], f32, tag="ss", name="ss")
            tgt = pp.tile([128, ni], f32, tag="tgt", name="tgt")
            u = pp.tile([128, ni], f32, tag="u", name="u")
            a_all = pp.tile([128, ni], f32, tag="a_all", name="a_all")
            a2_all = pp.tile([128, ni], f32, tag="a2_all", name="a2_all")
            loc = pp.tile([128, ni], f32, tag="loc", name="loc")
            ones = pp.tile([128, 1], f32, tag="ones", name="ones")

            # xT resident in SBUF (fp8 straight from HBM) -- gates the
            # first matmuls, so issue these DMAs first
            for g in range(nk8):
                nc.sync.dma_start(
                    xtb[g][:],
                    xtb_d[g * 256:(g + 1) * 256, :].rearrange(
                        "(s p) n -> p s n", s=2))

            # W staging for one chunk. A dma_start costs ~650ns of issue
            # time on its sequencer, so steady-state uses ONE wide DMA
            # per contraction half, alternating the two doorbells between
            # the idle GpSimd queue and the Sync queue. Chunk 0 is instead
            # split per c-tile so its pieces land on many DMA queues in
            # parallel (first-matmul latency).
            def stage_chunk(ci, narrow=False):
                j0, ws = chunks[ci]
                gw = sum(ws)
                c0 = j0 * ct
                wbt = {}
                for g in range(nk8):
                    wb = wbp.tile([128, 2, 4 * ct], fp8, tag="wb", name="wb")
                    eng = nc.gpsimd if g == 0 else nc.sync
                    if narrow:
                        co = 0
                        for w in ws:
                            eng.dma_start(
                                wb[:, :, co:co + w],
                                wt[g * 256:(g + 1) * 256,
                                   c0 + co:c0 + co + w].rearrange(
                                    "(s p) c -> p s c", s=2))
                            co += w
                    else:
                        eng.dma_start(
                            wb[:, :, :gw],
                            wt[g * 256:(g + 1) * 256,
                               c0:c0 + gw].rearrange("(s p) c -> p s c",
                                                     s=2))
                    wbt[g] = wb
                return wbt

            # stage chunk 0 and the x tiles first (they gate the first
            # matmuls / the first Exp's a_all), deeper W prefetch after
            staged = {0: stage_chunk(0, narrow=True)}
            for i in range(ni):
                nc.sync.dma_start(xa[i][:], x_nat[i * 128:(i + 1) * 128, :])
            for ci in range(1, prefetch_chunks):
                staged[ci] = stage_chunk(ci)

            nc.vector.memset(ones[:], 1.0)

            # norms: a[n] = S / ||x_n|| = exp(-0.5 * ln(ss / S^2)); Ln+Exp
            # share one ScalarE table set so there is no table thrashing
            for i in range(ni):
                sq = scp.tile([128, d], f32, tag="sq", name="sq")
                nc.vector.tensor_mul(sq[:], xa[i][:], xa[i][:])
                nc.vector.reduce_sum(ss[:, i:i + 1], sq[:], axis=AX.X)
            nc.scalar.activation(u[:], ss[:], AF.Ln, scale=1.0 / (S * S))
            nc.scalar.activation(a_all[:], u[:], AF.Exp, scale=-0.5)
            nc.vector.tensor_scalar_mul(a2_all[:], a_all[:], EXP_A)

            # target-logit work for n-tile i (DVE mul + row reduce)
            def tgt_work(i):
                wla = sp.tile([128, d], bf16, tag="wla", name="wla")
                nc.sync.dma_start(wla[:], wl[i * 128:(i + 1) * 128, :])
                pr = scp.tile([128, d], f32, tag="pr", name="pr")
                nc.vector.tensor_mul(pr[:], xa[i][:], wla[:])
                nc.vector.reduce_sum(tgt[:, i:i + 1], pr[:], axis=AX.X)

            # main loop: 49 chunks x 4 n-tiles. One PSUM group ([128,
            # up to 2048], 4 banks) per (chunk, i); a single wide Exp
            # activation with fused row-sum accumulator consumes the
            # group, except every dve_every-th group which goes to
            # VectorE via the Schraudolph fast-exp bit trick.
            with tc.tile_pool(name="psum", bufs=2, space="PSUM") as psp:
                for ci, (j0, ws) in enumerate(chunks):
                    pf = ci + prefetch_chunks
                    if pf < n_chunks:
                        staged[pf] = stage_chunk(pf)
                    wbt = staged.pop(ci)
                    gw = sum(ws)
                    for i in range(ni):
                        ps = psp.tile([128, psg_w], f32, tag="ps", name="ps")
                        for g in range(nk8):
                            lhs = xtb[g][:, :, i * 128:(i + 1) * 128]
                            co = 0
                            for jc, w in enumerate(ws):
                                nc.tensor.matmul(
                                    ps[:, co:co + w], lhs,
                                    wbt[g][:, :, co:co + w],
                                    start=(g == 0), stop=(g == nk8 - 1),
                                    perf_mode=(
                                        mybir.MatmulPerfMode.DoubleRow))
                                co += w
                        col = i * n_chunks + ci
                        gi = ci * ni + i
                        if gi % dve_every == dve_every - 1:
                            # whole-group DVE fast-exp (Schraudolph)
                            ti = scp.tile([128, psg_w], i32, tag="ti",
                                          name="ti")
                            nc.vector.tensor_scalar(
                                out=ti[:, :gw], in0=ps[:, :gw],
                                scalar1=a2_all[:, i:i + 1], scalar2=EXP_B,
                                op0=ALU.mult, op1=ALU.add)
                            nc.vector.reduce_sum(parts[:, col:col + 1],
                                                 ti[:, :gw].bitcast(f32),
                                                 axis=AX.X)
                        else:
                            es = scp.tile([128, psg_w], bf16, tag="es",
                                          name="es")
                            nc.scalar.activation(
                                es[:, :gw], ps[:, :gw], AF.Exp,
                                scale=a_all[:, i:i + 1],
                                accum_out=parts[:, col:col + 1])
                        # interleave tgt work across the first chunks
                        if 1 <= ci <= ni and i == 2:
                            tgt_work(ci - 1)

            # per-sample local exp-sum over this core's chunks
            for i in range(ni):
                nc.vector.reduce_sum(
                    loc[:, i:i + 1],
                    parts[:, i * n_chunks:(i + 1) * n_chunks],
                    axis=AX.X)

            # epilogue: per-sample v = log(den) - S*tgt; den = loc
            # - exp(S*tgt) + exp(S*tgt - SM)
            t1 = pp.tile([128, ni], f32, tag="t1", name="t1")
            e1 = pp.tile([128, ni], f32, tag="e1", name="e1")
            e2 = pp.tile([128, ni], f32, tag="e2", name="e2")
            den = pp.tile([128, ni], f32, tag="den", name="den")
            lg = pp.tile([128, ni], f32, tag="lg", name="lg")
            v = pp.tile([128, ni], f32, tag="v", name="v")
            rowv = pp.tile([128, 1], f32, tag="rowv", name="rowv")
            trow = pp.tile([128, 1], f32, tag="trow", name="trow")
            res = pp.tile([1, 1], f32, tag="res", name="res")

            nc.vector.tensor_mul(t1[:], a_all[:], tgt[:])     # S * tgt cosine
            nc.scalar.activation(e2[:], t1[:], AF.Exp)
            # exp(t1 - SM) == exp(t1) * exp(-SM)
            nc.vector.tensor_scalar_mul(e1[:], e2[:], float(np.exp(-SM)))
            nc.vector.tensor_sub(den[:], loc[:], e2[:])
            nc.vector.tensor_add(den[:], den[:], e1[:])
            nc.scalar.activation(lg[:], den[:], AF.Ln)
            nc.vector.tensor_sub(v[:], lg[:], t1[:])
            nc.vector.reduce_sum(rowv[:], v[:], axis=AX.X)

            # AllReduce the per-partition partial sums (512 B), then
            # partition-sum via ones-matmul; every core emits the mean
            cc_in = dp.tile([128, 1], f32, name="cc_in")
            cc_out = dp.tile([128, 1], f32, addr_space="Shared",
                             name="cc_out")
            nc.sync.dma_start(cc_in[:], rowv[:])
            nc.gpsimd.collective_compute(
                "AllReduce", ALU.add,
                replica_groups=[list(range(n_cores))],
                ins=[cc_in[:]], outs=[cc_out[:]])
            nc.sync.dma_start(trow[:], cc_out[:])

            with tc.tile_pool(name="psum1", bufs=1, space="PSUM") as psp1:
                pss = psp1.tile([1, 1], f32, tag="pss", name="pss")
                nc.tensor.matmul(pss[:], trow[:], ones[:], start=True,
                                 stop=True)
                nc.vector.tensor_scalar_mul(res[:], pss[:],
                                            1.0 / (n * n_cores))
                nc.vector.tensor_scalar_add(res[:], res[:], SM)
            nc.sync.dma_start(out[:], res[:])

    nc.compile()
    return nc


def in_maps(x, W, labels, n_cores=N_CORES):
    x = np.ascontiguousarray(np.asarray(x, dtype=np.float32))
    W = np.ascontiguousarray(np.asarray(W, dtype=np.float32))
    lab = np.asarray(labels).astype(np.int64)
    wt = np.ascontiguousarray(W.T.astype(np_fp8))       # shared by all cores
    wlg = W[lab].astype(np_bf16)                        # [N, D]
    maps = []
    ns = x.shape[0] // n_cores
    for c in range(n_cores):
        xs = x[c * ns:(c + 1) * ns]
        maps.append({
            "x_nat": np.ascontiguousarray(xs.astype(np_bf16)),
            "xtb": np.ascontiguousarray(xs.T.astype(np_fp8)),
            "wl": np.ascontiguousarray(wlg[c * ns:(c + 1) * ns]),
            "wt": wt,
        })
    return maps


_CACHE = {}


def _get_nc():
    if "nc" not in _CACHE:
        _CACHE["nc"] = build()
    return _CACHE["nc"]


def kernel(x, W, labels):
    nc = _get_nc()
    res = run_bass_kernel_spmd(nc, in_maps(x, W, labels),
                               core_ids=list(range(N_CORES)))
    val = np.asarray(res.results[0]["out"], dtype=np.float32)
    return val.reshape(())
